# revision 1
# baseline (speedup 1.0000x reference)
"""GAT (2-layer + classifier) Trainium2 Bass kernel, 8-core SPMD.

Sharding: destination nodes (and their incoming edges, sorted by dst) are
sharded across 8 cores; projected node features are replicated via AllGather;
per-node softmax numerator and denominator are accumulated with the weighted
message matmul (ones-column trick), so no cross-core reduction is needed.

v5 design (on top of v4's fp8 rows / streamed static one-hots / W-augment):
- Each layer's AllGather is split in two (tiles 0..24 -> table A, 25..48 ->
  table B).  Source windows are (core, pos<3200) -> row of table A else B, so
  window-A gathers only depend on collective A and overlap collective B and
  the producer compute of the next layer.
- Self-loop edges live in a dedicated last chunk per tile filled by ONE
  contiguous static DMA from the core's own agin rows -- no gather
  descriptors (the Pool-engine descriptor generation is the critical path).
- relu on the scalar engine; the per-tile softmax scale is issued in two
  halves so the message matmuls start earlier.
"""

import os
import sys

import numpy as np

sys.path.insert(0, "/opt/trn_rl_repo")

# ---------------- problem constants (hardcoded, from the GAT spec) ---------
N_NODES = 50000
N_EDGES = 800000
IN_DIM = 256
HID = 128
HEADS = 3
N_CLASSES = 40
HC = HEADS * HID  # 384
NEG_SLOPE = 0.2

NCORES = 8
NPC = N_NODES // NCORES  # 6250 nodes per core
TILE = 128
NTILES = (NPC + TILE - 1) // TILE  # 49 (last tile has 106 rows)
TILA = 25  # tiles 0..24 -> table A
NPA = TILA * TILE  # 3200 rows/core in table A
NPB = NPC - NPA  # 3050 rows/core in table B
WINA = NCORES * NPA  # 25600
WINB = NCORES * NPB  # 24400
MMN = 387  # matmul N: 3*(128+1), interleaved [h|1] blocks
AUGC = 390  # augmented weight cols: h(384) + asrc(3) + adst(3)
FP8 = os.environ.get("GAT_FP8", "1") == "1"
FP8SEQ = os.environ.get("GAT_FP8SEQ", "1") == "1"
ROWE = 512 if FP8 else 448  # gather row elems; bytes must be %256
ASRCB = 194 if FP8 else None  # bf16 idx of asrc within bitcast row (fp8)
ACT_HEAD = os.environ.get("GAT_ACT_HEAD", "1") == "1"  # head 2 scale on Act
RELU_ACT = os.environ.get("GAT_RELU_ACT", "1") == "1"
SPLIT_SCALE = os.environ.get("GAT_SPLIT_SCALE", "1") == "1"
SELF_CHUNK = os.environ.get("GAT_SELF_CHUNK", "1") == "1"

_CACHE = {}


def _round_up(x, m):
    return (x + m - 1) // m * m


def _bf16(a):
    import ml_dtypes

    return np.asarray(a, np.float32).astype(ml_dtypes.bfloat16)


# =========================================================================
# Host-side preprocessing: edge sort / shard / pad, idx + static one-hots
# =========================================================================
def _preprocess(edge_index):
    import ml_dtypes

    src = np.asarray(edge_index[0], dtype=np.int64)
    dst = np.asarray(edge_index[1], dtype=np.int64)
    # appended self loops go to the dedicated static chunk; natural (i,i)
    # edges (if any) stay in the normal gather lists

    core = dst // NPC
    rel = dst - core * NPC
    tile_i = rel // TILE
    spos = src % NPC
    win = (spos >= NPA).astype(np.int64)
    srow = np.where(win == 0, (src // NPC) * NPA + spos, (src // NPC) * NPB + spos - NPA)
    key = (core * NTILES + tile_i) * 2 + win
    order = np.argsort(key, kind="stable")
    srow = srow[order]
    rel = rel[order]

    counts = np.bincount(key[order], minlength=NCORES * NTILES * 2)
    starts = np.zeros(NCORES * NTILES * 2 + 1, np.int64)
    np.cumsum(counts, out=starts[1:])
    cnt = counts.reshape(NCORES, NTILES, 2)

    p0 = _round_up(cnt[:, :, 0].max(axis=0), 128)  # [NTILES] padded win0 len
    p1 = _round_up(cnt[:, :, 1].max(axis=0), 128)
    c_t = (p0 + p1) // 128 + 1  # chunks per tile (+1 self chunk)
    T0 = int(p0.sum())
    T1 = int(p1.sum())
    CTOT = int(c_t.sum())
    o0 = np.concatenate([[0], np.cumsum(p0)])  # idx elem offsets
    o1 = np.concatenate([[0], np.cumsum(p1)])
    oc = np.concatenate([[0], np.cumsum(c_t)])  # chunk offsets

    IDX0 = np.zeros((NCORES, 128, T0 // 16), np.int16)
    IDX1 = np.zeros((NCORES, 128, T1 // 16), np.int16)
    seqdt = ml_dtypes.float8_e4m3 if FP8SEQ else ml_dtypes.bfloat16
    ISEQ = np.zeros((NCORES, 128, CTOT * 128), seqdt)
    ISEQT = np.zeros((NCORES, 128, CTOT * 128), seqdt)
    d128 = np.arange(128)

    def wrap16(a):  # idx i -> [i%16, i//16], replicated to 128 partitions
        w = a.reshape(-1, 16).T
        return np.tile(w, (8, 1))

    for r in range(NCORES):
        for t in range(NTILES):
            c = c_t[t]
            nedge = 128 * (c - 1)
            drel_tile = np.full(nedge, -1.0, np.float32)
            for w, (P, O, IDX, off_in) in enumerate(
                ((p0, o0, IDX0, 0), (p1, o1, IDX1, p0[t]))
            ):
                k = (r * NTILES + t) * 2 + w
                s, e = starts[k], starts[k + 1]
                n = e - s
                idxs = np.zeros(P[t], np.int16)
                if n > 0:
                    idxs[:n] = srow[s:e].astype(np.int16)
                    idxs[n:] = idxs[n - 1]
                    drel_tile[off_in : off_in + n] = (rel[s:e] - t * TILE).astype(
                        np.float32
                    )
                if P[t] > 0:
                    IDX[r, :, O[t] // 16 : (O[t] + P[t]) // 16] = wrap16(idxs)
            dt = drel_tile.reshape(c - 1, 128)  # [chunk, edge-slot]
            oh = dt[:, :, None] == d128[None, None, :]  # [c-1, e, d]
            rt = min(TILE, NPC - t * TILE)
            eye = (d128[:, None] == d128[None, :]) & (d128[:, None] < rt)
            oh = np.concatenate([oh, eye[None]], axis=0)  # [c, e, d]
            sl = slice(oc[t] * 128, (oc[t] + c) * 128)
            ISEQ[r, :, sl] = (
                oh.transpose(1, 0, 2).reshape(128, c * 128).astype(seqdt)
            )
            ISEQT[r, :, sl] = (
                oh.transpose(2, 0, 1).reshape(128, c * 128).astype(seqdt)
            )

    sched = dict(
        p0=[int(v) for v in p0],
        p1=[int(v) for v in p1],
        c_t=[int(v) for v in c_t],
        o0=[int(v) for v in o0],
        o1=[int(v) for v in o1],
        oc=[int(v) for v in oc],
        T0=T0,
        T1=T1,
        CTOT=CTOT,
    )
    return sched, IDX0, IDX1, ISEQ, ISEQT


def _augment(W, att_src, att_dst):
    """[K, 384] -> [K, 390] with per-head att_src/att_dst projections."""
    W = np.asarray(W, np.float32)
    cols = [W]
    for att in (att_src, att_dst):
        a = np.zeros((W.shape[0], HEADS), np.float32)
        for h in range(HEADS):
            a[:, h] = W[:, HID * h : HID * (h + 1)] @ np.asarray(att[h], np.float32)
        cols.append(a)
    return np.concatenate(cols, axis=1)  # [K, 390]


def _shared_inputs(W1, att_src1, att_dst1, b1, W2, att_src2, att_dst2, b2, outW, outb):
    f = np.float32
    return {
        "W1A": _bf16(_augment(W1, att_src1, att_dst1)),  # [256, 390]
        "W2A": _bf16(_augment(W2, att_src2, att_dst2)),  # [384, 390]
        "OUTW": _bf16(outW),  # [384, 40]
        "B1R": _bf16(np.tile(np.asarray(b1, f)[None, :], (128, 1))),
        "B2R": _bf16(np.tile(np.asarray(b2, f)[None, :], (128, 1))),
        "OUTBR": np.tile(np.asarray(outb, f)[None, :], (128, 1)),
        "IDENT": _bf16(np.eye(128, dtype=f)),
    }


# =========================================================================
# Bass program
# =========================================================================
def _build_program(sched):
    from contextlib import ExitStack

    import concourse.bass as bass
    import concourse.mybir as mybir
    import concourse.tile as tile
    from concourse import bacc

    f32 = mybir.dt.float32
    bf16 = mybir.dt.bfloat16
    fp8 = mybir.dt.float8e4
    i16 = mybir.dt.int16
    gdt = fp8 if FP8 else f32
    AF = mybir.ActivationFunctionType
    OP = mybir.AluOpType
    AP = bass.AP

    p0, p1, c_t = sched["p0"], sched["p1"], sched["c_t"]
    o0, o1, oc = sched["o0"], sched["o1"], sched["oc"]
    T0, T1, CTOT = sched["T0"], sched["T1"], sched["CTOT"]

    nc = bacc.Bacc(
        "TRN2",
        target_bir_lowering=False,
        debug=False,
        enable_asserts=False,
        num_devices=NCORES,
        num_swdge_queues=2,
        dynamic_dma_scratch_size=int(os.environ.get("GAT_DMA_SCRATCH", 16384)),
    )

    # ---- I/O ----
    XTT = nc.dram_tensor("XTT", [NTILES * IN_DIM, TILE], bf16, kind="ExternalInput")
    IDX0 = nc.dram_tensor("IDX0", [128, T0 // 16], i16, kind="ExternalInput")
    IDX1 = nc.dram_tensor("IDX1", [128, T1 // 16], i16, kind="ExternalInput")
    sdt = fp8 if FP8SEQ else bf16
    ISEQ = nc.dram_tensor("ISEQ", [128, CTOT * 128], sdt, kind="ExternalInput")
    ISEQT = nc.dram_tensor("ISEQT", [128, CTOT * 128], sdt, kind="ExternalInput")
    W1A = nc.dram_tensor("W1A", [IN_DIM, AUGC], bf16, kind="ExternalInput")
    W2A = nc.dram_tensor("W2A", [HC, AUGC], bf16, kind="ExternalInput")
    OUTW = nc.dram_tensor("OUTW", [HC, N_CLASSES], bf16, kind="ExternalInput")
    B1R = nc.dram_tensor("B1R", [128, HC], bf16, kind="ExternalInput")
    B2R = nc.dram_tensor("B2R", [128, HC], bf16, kind="ExternalInput")
    OUTBR = nc.dram_tensor("OUTBR", [128, N_CLASSES], f32, kind="ExternalInput")
    IDENT = nc.dram_tensor("IDENT", [128, 128], bf16, kind="ExternalInput")
    OUT = nc.dram_tensor("OUT", [NPC, N_CLASSES], f32, kind="ExternalOutput")

    def strided3(ap2d, start, step, count):
        # [128, N] -> [128, count] picking cols start, start+step, ...
        base = ap2d[:, start : start + 1]
        return AP(base.tensor, base.offset, [base.ap[0], [step, count]])

    def seg_view(ap2d, nseg, seglen, stride):
        # [128, N] -> [128, nseg, seglen] with segment stride `stride`
        return AP(ap2d.tensor, ap2d.offset, [ap2d.ap[0], [stride, nseg], [1, seglen]])

    with tile.TileContext(nc) as tc, ExitStack() as ctx:
        cpool = ctx.enter_context(tc.tile_pool(name="cpool", bufs=1))
        dram = ctx.enter_context(tc.tile_pool(name="dram", bufs=1, space="DRAM"))
        gpool = ctx.enter_context(tc.tile_pool(name="gpool", bufs=2))
        wpool = ctx.enter_context(tc.tile_pool(name="wpool", bufs=2))
        ppool = ctx.enter_context(tc.tile_pool(name="ppool", bufs=2, space="PSUM"))
        apool = ctx.enter_context(tc.tile_pool(name="apool", bufs=3, space="PSUM"))

        # resident constants
        idx0_sb = cpool.tile_from(IDX0.ap())
        idx1_sb = cpool.tile_from(IDX1.ap())
        b1r_sb = cpool.tile_from(B1R.ap())
        b2r_sb = cpool.tile_from(B2R.ap())
        outbr_sb = cpool.tile_from(OUTBR.ap())
        ident_sb = cpool.tile_from(IDENT.ap())
        w1_sb = [
            cpool.tile_from(W1A.ap()[128 * k : 128 * (k + 1), :], name=f"w1_{k}")
            for k in range(2)
        ]
        w2_sb = [
            cpool.tile_from(W2A.ap()[128 * k : 128 * (k + 1), :], name=f"w2_{k}")
            for k in range(3)
        ]
        outw_sb = [
            cpool.tile_from(OUTW.ap()[128 * k : 128 * (k + 1), :], name=f"outw_{k}")
            for k in range(3)
        ]
        adst = cpool.tile([128, NTILES * HEADS], bf16)  # per-layer a_dst per tile

        aginA1 = dram.tile([NPA, ROWE], gdt)
        aginB1 = dram.tile([NPB, ROWE], gdt)
        hextA1 = dram.tile([WINA, ROWE], gdt, addr_space="Shared")
        hextB1 = dram.tile([WINB, ROWE], gdt, addr_space="Shared")
        aginA2 = dram.tile([NPA, ROWE], gdt)
        aginB2 = dram.tile([NPB, ROWE], gdt)
        hextA2 = dram.tile([WINA, ROWE], gdt, addr_space="Shared")
        hextB2 = dram.tile([WINB, ROWE], gdt, addr_space="Shared")

        def rows_of(t):
            return min(TILE, NPC - t * TILE)

        def agin_slice(aginA, aginB, t):
            r = rows_of(t)
            if t < TILA:
                return aginA[TILE * t : TILE * t + r, :]
            return aginB[TILE * (t - TILA) : TILE * (t - TILA) + r, :]

        def pack_row(t, src_psum):
            """psum [128, 390] = [h(384)|asrc(3)|adst(3)] -> table row tile."""
            row = wpool.tile([128, ROWE], gdt, tag="row")
            nc.gpsimd.memset(row[:, MMN:ROWE], 0.0)
            nc.vector.tensor_copy(
                seg_view(row, HEADS, HID, HID + 1), seg_view(src_psum, HEADS, HID, HID)
            )
            nc.vector.memset(strided3(row, HID, HID + 1, HEADS), 1.0)
            if FP8:
                rb = row[:].bitcast(bf16)  # [128, 256]
                nc.vector.tensor_copy(
                    rb[:, ASRCB : ASRCB + 3], src_psum[:, HC : HC + 3]
                )
            else:
                nc.vector.tensor_copy(row[:, MMN : MMN + 3], src_psum[:, HC : HC + 3])
            nc.vector.tensor_copy(
                adst[:, HEADS * t : HEADS * (t + 1)], src_psum[:, HC + 3 : HC + 6]
            )
            return row

        def phase1_tile(t):
            h1_ps = apool.tile([128, AUGC], f32, tag="acc")
            for k in range(2):
                xk = wpool.tile([128, 128], bf16, tag="xk")
                nc.sync.dma_start(
                    out=xk[:],
                    in_=XTT.ap()[IN_DIM * t + 128 * k : IN_DIM * t + 128 * (k + 1), :],
                )
                nc.tensor.matmul(
                    h1_ps[:], lhsT=xk[:], rhs=w1_sb[k][:], start=(k == 0), stop=(k == 1)
                )
            row = pack_row(t, h1_ps)
            nc.sync.dma_start(out=agin_slice(aginA1, aginB1, t), in_=row[: rows_of(t), :])

        def allgather(agin, hext):
            nc.gpsimd.collective_compute(
                "AllGather",
                mybir.AluOpType.bypass,
                replica_groups=[list(range(NCORES))],
                ins=[agin[:]],
                outs=[hext[:]],
            )

        # ---------------- Phase 1: h1 = x @ W1A, pack, A/B AllGather --------
        for t in range(TILA):
            phase1_tile(t)
        allgather(aginA1, hextA1)
        for t in range(TILA, NTILES):
            phase1_tile(t)
        allgather(aginB1, hextB1)

        # ---------------- Edge pass (shared for both layers) ----------------
        def edge_pass(t, hextA, hextB, aginA, aginB):
            c = c_t[t]
            q0 = p0[t] // 128
            q01 = q0 + p1[t] // 128
            G = gpool.tile([128, c, ROWE], gdt, tag="G")
            if p0[t] > 0:
                nc.gpsimd.dma_gather(
                    out_ap=G[:, :q0, :],
                    in_ap=hextA[0:WINA, :],
                    idxs_ap=idx0_sb[:, o0[t] // 16 : (o0[t] + p0[t]) // 16],
                    num_idxs=p0[t],
                    num_idxs_reg=p0[t],
                    elem_size=ROWE,
                    queue_num=0,
                    single_packet=False,
                )
            if p1[t] > 0:
                nc.gpsimd.dma_gather(
                    out_ap=G[:, q0:q01, :],
                    in_ap=hextB[0:WINB, :],
                    idxs_ap=idx1_sb[:, o1[t] // 16 : (o1[t] + p1[t]) // 16],
                    num_idxs=p1[t],
                    num_idxs_reg=p1[t],
                    elem_size=ROWE,
                    queue_num=1,
                    single_packet=False,
                )
            # self-loop chunk: contiguous read of this core's own packed rows
            r = rows_of(t)
            if r < TILE:
                nc.vector.memset(G[:, c - 1, :], 0.0)
            nc.sync.dma_start(
                out=G[:r, c - 1, :], in_=agin_slice(aginA, aginB, t)
            )
            # stream the static one-hot blocks for this tile
            iseq = wpool.tile([128, c, 128], sdt, tag="iseq")
            nc.sync.dma_start(
                out=iseq[:], in_=ISEQ.ap()[:, oc[t] * 128 : (oc[t] + c) * 128]
            )
            iseqT = wpool.tile([128, c, 128], sdt, tag="iseqT")
            nc.sync.dma_start(
                out=iseqT[:], in_=ISEQT.ap()[:, oc[t] * 128 : (oc[t] + c) * 128]
            )
            # a_dst per edge: dcol[:, ci, :] = iseqT_ci^T @ adst_t
            dcol_ps = ppool.tile([128, c, HEADS], f32, tag="dcol")
            for ci in range(c):
                nc.tensor.matmul(
                    dcol_ps[:, ci, :],
                    lhsT=iseqT[:, ci, :],
                    rhs=adst[:, HEADS * t : HEADS * (t + 1)],
                    start=True,
                    stop=True,
                )
            # alpha / leaky relu / exp   [128, c, 3]
            if FP8:
                gb = G[:].bitcast(bf16)  # [128, c, 256]
                asrcv = gb[:, :, ASRCB : ASRCB + 3]
            else:
                asrcv = G[:, :, MMN : MMN + 3]
            alpha = wpool.tile([128, c, HEADS], bf16, tag="alpha")
            nc.vector.tensor_tensor(
                out=alpha[:], in0=asrcv, in1=dcol_ps[:], op=OP.add
            )
            nc.vector.scalar_tensor_tensor(
                out=alpha[:], in0=alpha[:], scalar=NEG_SLOPE, in1=alpha[:],
                op0=OP.mult, op1=OP.max,
            )
            ex = wpool.tile([128, c, HEADS], f32, tag="ex")
            nc.scalar.activation(ex[:], alpha[:], AF.Exp)
            # Gs = G[0:387] * ex (per 129-col head block; ones cols -> denom)
            Gs = gpool.tile([128, c, MMN], bf16, tag="Gs")
            nh = 2 if ACT_HEAD else 3
            ch = c // 2 if SPLIT_SCALE else 0
            for a, b in (((0, ch), (ch, c)) if SPLIT_SCALE else ((0, c),)):
                n = b - a
                gt, st, et = G[:], Gs[:], ex[:]
                g4 = AP(gt.tensor, gt.offset + a * ROWE,
                        [gt.ap[0], [ROWE, n], [HID + 1, nh], [1, HID + 1]])
                s4 = AP(st.tensor, st.offset + a * MMN,
                        [st.ap[0], [MMN, n], [HID + 1, nh], [1, HID + 1]])
                e4 = AP(et.tensor, et.offset + a * HEADS,
                        [et.ap[0], [HEADS, n], [1, nh], [0, HID + 1]])
                nc.vector.tensor_tensor(out=s4, in0=g4, in1=e4, op=OP.mult)
            if ACT_HEAD:
                for ci in range(c):
                    nc.scalar.activation(
                        Gs[:, ci, 2 * (HID + 1) : MMN],
                        G[:, ci, 2 * (HID + 1) : MMN],
                        AF.Copy,
                        scale=ex[:, ci, 2:3],
                    )
            # weighted segment sum (and denominators via the ones columns)
            out_ps = apool.tile([128, MMN], f32, tag="acc")
            for ci in range(c):
                nc.tensor.matmul(
                    out_ps[:],
                    lhsT=iseq[:, ci, :],
                    rhs=Gs[:, ci, :],
                    start=(ci == 0),
                    stop=(ci == c - 1),
                )
            return out_ps

        def normalize(out_ps, brep_sb):
            """h = relu(out/denom + bias)  -> [128, 384] bf16 sbuf tile"""
            tmp3 = wpool.tile([128, HEADS], f32, tag="tmp3")
            nc.vector.tensor_scalar_add(
                tmp3[:], strided3(out_ps, HID, HID + 1, HEADS), 1e-16
            )
            r3 = wpool.tile([128, HEADS], f32, tag="r3")
            nc.vector.reciprocal(r3[:], tmp3[:])
            h2 = wpool.tile([128, HC], bf16, tag="h2")
            for h in range(HEADS):
                nc.vector.scalar_tensor_tensor(
                    out=h2[:, HID * h : HID * (h + 1)],
                    in0=out_ps[:, (HID + 1) * h : (HID + 1) * h + HID],
                    scalar=r3[:, h : h + 1],
                    in1=brep_sb[:, HID * h : HID * (h + 1)],
                    op0=OP.mult,
                    op1=OP.add,
                )
            if RELU_ACT:
                nc.scalar.activation(h2[:], h2[:], AF.Relu)
            else:
                nc.vector.tensor_scalar_max(h2[:], h2[:], 0.0)
            return h2

        # ---------------- Phase 2: edge pass L1 + entry L2 ------------------
        limit = int(os.environ.get("GAT_LIMIT_TILES", NTILES))

        def phase2_tile(t):
            out_ps = edge_pass(t, hextA1, hextB1, aginA1, aginB1)
            h2 = normalize(out_ps, b1r_sb)
            h3_ps = apool.tile([128, AUGC], f32, tag="acc")
            for k in range(3):
                tp = ppool.tile([128, 128], bf16, tag="sq")
                nc.tensor.transpose(tp[:], h2[:, 128 * k : 128 * (k + 1)], ident_sb[:])
                h2T = wpool.tile([128, 128], bf16, tag="h2T", bufs=3)
                nc.scalar.activation(h2T[:], tp[:], AF.Copy)
                nc.tensor.matmul(
                    h3_ps[:], lhsT=h2T[:], rhs=w2_sb[k][:], start=(k == 0), stop=(k == 2)
                )
            row = pack_row(t, h3_ps)
            nc.sync.dma_start(out=agin_slice(aginA2, aginB2, t), in_=row[: rows_of(t), :])

        for t in range(min(TILA, limit)):
            phase2_tile(t)
        allgather(aginA2, hextA2)
        for t in range(TILA, min(NTILES, limit)):
            phase2_tile(t)
        allgather(aginB2, hextB2)

        # ---------------- Phase 3: edge pass L2 + classifier ----------------
        for t in range(min(NTILES, limit)):
            out_ps = edge_pass(t, hextA2, hextB2, aginA2, aginB2)
            h3 = normalize(out_ps, b2r_sb)
            cls_ps = ppool.tile([128, N_CLASSES], f32, tag="dcol")
            for k in range(3):
                tp = ppool.tile([128, 128], bf16, tag="sq")
                nc.tensor.transpose(tp[:], h3[:, 128 * k : 128 * (k + 1)], ident_sb[:])
                h3T = wpool.tile([128, 128], bf16, tag="h2T", bufs=3)
                nc.scalar.activation(h3T[:], tp[:], AF.Copy)
                nc.tensor.matmul(
                    cls_ps[:], lhsT=h3T[:], rhs=outw_sb[k][:], start=(k == 0), stop=(k == 2)
                )
            outt = wpool.tile([128, N_CLASSES], f32, tag="outt")
            nc.vector.tensor_tensor(out=outt[:], in0=cls_ps[:], in1=outbr_sb[:], op=OP.add)
            r = rows_of(t)
            nc.sync.dma_start(out=OUT.ap()[TILE * t : TILE * t + r, :], in_=outt[:r, :])

    nc.compile()
    return nc


# =========================================================================
# entry point
# =========================================================================
def _prepare(inputs):
    """Build (cached) program + per-core input maps from FULL inputs."""
    import ml_dtypes

    x = np.asarray(inputs["x"], np.float32)
    edge_index = np.asarray(inputs["edge_index"])

    key = "prog"
    if key not in _CACHE:
        sched, IDX0, IDX1, ISEQ, ISEQT = _preprocess(edge_index)
        nc = _build_program(sched)
        _CACHE[key] = (sched, IDX0, IDX1, ISEQ, ISEQT, nc)
    sched, IDX0, IDX1, ISEQ, ISEQT, nc = _CACHE[key]

    shared = _shared_inputs(
        inputs["W1"], inputs["att_src1"], inputs["att_dst1"], inputs["b1"],
        inputs["W2"], inputs["att_src2"], inputs["att_dst2"], inputs["b2"],
        inputs["outW"], inputs["outb"],
    )

    in_maps = []
    for r in range(NCORES):
        xs = x[r * NPC : (r + 1) * NPC]  # [NPC, 256]
        xtt = np.zeros((NTILES * IN_DIM, TILE), ml_dtypes.bfloat16)
        for t in range(NTILES):
            rt = min(TILE, NPC - t * TILE)
            xtt[IN_DIM * t : IN_DIM * (t + 1), :rt] = (
                xs[TILE * t : TILE * t + rt].T.astype(ml_dtypes.bfloat16)
            )
        m = dict(shared)
        m["XTT"] = xtt
        m["IDX0"] = IDX0[r]
        m["IDX1"] = IDX1[r]
        m["ISEQ"] = ISEQ[r]
        m["ISEQT"] = ISEQT[r]
        in_maps.append(m)
    return nc, in_maps


def _assemble(results):
    return np.concatenate([results[r]["OUT"] for r in range(NCORES)], axis=0)


def kernel(**inputs):
    nc, in_maps = _prepare(inputs)

    from concourse.bass_utils import run_bass_kernel_spmd

    res = run_bass_kernel_spmd(nc, in_maps, core_ids=list(range(NCORES)))
    return _assemble(res.results)


if __name__ == "__main__":
    sys.path.insert(0, os.path.dirname(os.path.abspath(__file__)))
    import reference

    inp = {k: np.asarray(v) for k, v in reference.setup_inputs().items()}
    got = kernel(**inp)
    exp = np.asarray(reference.reference(**reference.setup_inputs()))
    err = np.abs(got - exp).max() / (np.abs(exp).max() + 1e-12)
    print("rel err:", err)

